# revision 1
# baseline (speedup 1.0000x reference)
"""CrossAttention Trainium2 Bass kernel.

Problem (hardcoded): B=16, Lq=Lk=2048, Dq=768, Dk=1024, fp32.
  q = query @ Wq + bq ; k = key @ Wk + bk ; v = key @ Wv + bv
  out = softmax(q k^T / sqrt(1024)) @ v

Sharding: data-parallel over batch, 2 batches per core on 8 cores.

Math simplifications (exact up to fp32 rounding):
  - bk shifts every score row by a constant (per query) -> cancels in softmax,
    so bk is dropped entirely.
  - softmax weights sum to 1, so bv passes through attention unchanged:
    add bv once to the final output instead of to v.
  - scores are bounded (|s|/32 < ~3) so exp() without max-subtraction is safe.

Per-core schedule (per batch):
  A) queryT via PE transposes; qT = Wq^T queryT (+bq) ; spill qT to DRAM.
  B1) keyT via PE transposes; kT = Wk^T keyT (SBUF resident); spill keyT.
  B2) v = keyT^T Wv (SBUF resident), streaming keyT back from DRAM.
  C) flash-style attention over Lq tiles of 256:
     scoresT = kT_chunk^T qT_tile (PSUM, 8 k-chunks), expT = exp(scores/32),
     out = sum_lk expT^T v (+ones-column trick for row sums via a separate
     N=1 matmul), normalize by reciprocal of sums, + bv, DMA out.

Matmul dtype: float32r (fp32 data, fast PE mode) by default; MM_DT knob
falls back to plain float32 if hardware numerics are insufficient.
"""

import os
import numpy as np

B, LQ, LK = 16, 2048, 2048
DQ, DK = 768, 1024
N_CORES = 8
BPC = B // N_CORES  # batches per core

MM_DT = os.environ.get("XATTN_MM_DT", "float32r")


def build_nc(bpc=BPC, lq=LQ, lk=LK, mm_dt=MM_DT, lq_t=256, c_t=512, reps=1):
    import concourse.bass as bass
    import concourse.mybir as mybir
    from concourse import bacc
    import concourse.tile as tile
    from concourse.masks import make_identity

    fp32 = mybir.dt.float32
    mdt = getattr(mybir.dt, mm_dt)
    KCQ = DQ // 128   # 6 contraction chunks for q projection
    KCK = DK // 128   # 8 contraction chunks for k/v projection + scores
    NLQ = lq // lq_t  # Lq tiles (projection phase)
    NLK = lk // 128   # Lk subtiles of 128
    LS = lq_t // 128  # Lq subtiles per tile (projection phase)
    NCQ = lq // c_t   # Lq tiles (attention phase)
    CS = c_t // 128   # Lq subtiles per attention tile

    nc = bacc.Bacc("TRN2")
    query = nc.dram_tensor("query", [bpc, lq, DQ], mdt, kind="ExternalInput")
    key = nc.dram_tensor("key", [bpc, lk, DK], mdt, kind="ExternalInput")
    Wq = nc.dram_tensor("Wq", [DQ, DK], mdt, kind="ExternalInput")
    bq = nc.dram_tensor("bq", [DK], fp32, kind="ExternalInput")
    Wk = nc.dram_tensor("Wk", [DK, DK], mdt, kind="ExternalInput")
    Wv = nc.dram_tensor("Wv", [DK, DK], mdt, kind="ExternalInput")
    bv = nc.dram_tensor("bv", [DK], fp32, kind="ExternalInput")
    out = nc.dram_tensor("out", [bpc, lq, DK], fp32, kind="ExternalOutput")
    vtag = nc.dram_tensor("variant_tag", [max(1, reps), 8], fp32, kind="ExternalInput")
    qT_dram = nc.dram_tensor("qT_scratch", [bpc, 128, KCK, lq], mdt, kind="Internal")
    keyT_dram = nc.dram_tensor("keyT_scratch", [bpc, 128, KCK, lk], mdt, kind="Internal")

    def mm(ps, lhsT, rhs, start, stop):
        nc.tensor.matmul(ps, lhsT, rhs, start=start, stop=stop)

    with tile.TileContext(nc) as tc:
        with (
            tc.tile_pool(name="const", bufs=1) as constp,
            tc.tile_pool(name="kT", bufs=1) as kTp,
            tc.tile_pool(name="v", bufs=1) as vp,
        ):
            ident_f32 = constp.tile([128, 128], fp32)
            make_identity(nc, ident_f32)
            if mdt == fp32:
                ident = ident_f32
            else:
                ident = constp.tile([128, 128], mdt)
                nc.vector.tensor_copy(ident, ident_f32)
            ones_col = constp.tile([128, 4], mdt)
            if mdt == fp32:
                nc.vector.memset(ones_col, 1.0)
            else:
                ones_f32 = constp.tile([128, 4], fp32)
                nc.vector.memset(ones_f32, 1.0)
                nc.vector.tensor_copy(ones_col, ones_f32)
            bq_sb = constp.tile([128, KCK], fp32)
            nc.sync.dma_start(bq_sb, bq.rearrange("(c p) -> p c", p=128))
            bv_rep = constp.tile([128, DK], fp32)
            nc.sync.dma_start(bv_rep, bv[None, :].partition_broadcast(128))
            vt_sb = constp.tile([1, 8], fp32)
            nc.sync.dma_start(vt_sb, vtag[0:1, :])

            for b in [bb for _ in range(reps) for bb in range(bpc)]:
                kT_sb = kTp.tile([128, KCK, lk], mdt)   # kT[dk, lk]
                v_sb = vp.tile([128, NLK, DK], mdt)     # v[lk, dk]

                # ---- Phase A: qT = Wq^T queryT + bq, spilled to DRAM ----
                with (
                    tc.tile_pool(name="qproj", bufs=2) as qp,
                    tc.tile_pool(name="wq", bufs=1) as wqp,
                    tc.tile_pool(name="qps", bufs=2, space="PSUM") as qps,
                ):
                    wq_sb = wqp.tile([128, KCQ, DK], mdt)
                    nc.sync.dma_start(wq_sb, Wq.rearrange("(c p) n -> p c n", p=128))
                    for t in range(NLQ):
                        qn = qp.tile([128, LS, DQ], mdt, tag="qnat")
                        nc.sync.dma_start(
                            qn,
                            query[b, t * lq_t:(t + 1) * lq_t, :].rearrange(
                                "(s p) d -> p s d", p=128
                            ),
                        )
                        qTt = qp.tile([128, KCQ, lq_t], mdt, tag="qTt")
                        for s in range(LS):
                            for kc in range(KCQ):
                                ps = qps.tile([128, 128], mdt, tag="tp")
                                nc.tensor.transpose(
                                    ps, qn[:, s, kc * 128:(kc + 1) * 128], ident
                                )
                                nc.vector.tensor_copy(
                                    qTt[:, kc, s * 128:(s + 1) * 128], ps
                                )
                        qTsb = qp.tile([128, KCK, lq_t], mdt, tag="qTsb")
                        for mc in range(KCK):
                            ps = qps.tile([128, lq_t], fp32, tag="mm")
                            for kc in range(KCQ):
                                mm(ps, wq_sb[:, kc, mc * 128:(mc + 1) * 128],
                                   qTt[:, kc, :], kc == 0, kc == KCQ - 1)
                            nc.vector.tensor_scalar_add(
                                qTsb[:, mc, :], ps, bq_sb[:, mc:mc + 1]
                            )
                        nc.sync.dma_start(
                            qT_dram[b, :, :, t * lq_t:(t + 1) * lq_t], qTsb
                        )

                # ---- Phase B1: keyT (spill) + kT resident ----
                with (
                    tc.tile_pool(name="kproj", bufs=1) as kp,
                    tc.tile_pool(name="wk", bufs=1) as wkp,
                    tc.tile_pool(name="kps", bufs=2, space="PSUM") as kps,
                ):
                    wk_sb = wkp.tile([128, KCK, DK], mdt)
                    nc.sync.dma_start(wk_sb, Wk.rearrange("(c p) n -> p c n", p=128))
                    for t in range(lk // 512):
                        kn = kp.tile([128, 4, DK], mdt, tag="knat")
                        nc.sync.dma_start(
                            kn,
                            key[b, t * 512:(t + 1) * 512, :].rearrange(
                                "(s p) d -> p s d", p=128
                            ),
                        )
                        kTt = kp.tile([128, KCK, 512], mdt, tag="kTt")
                        for s in range(4):
                            for kc in range(KCK):
                                ps = kps.tile([128, 128], mdt, tag="tp")
                                nc.tensor.transpose(
                                    ps, kn[:, s, kc * 128:(kc + 1) * 128], ident
                                )
                                nc.vector.tensor_copy(
                                    kTt[:, kc, s * 128:(s + 1) * 128], ps
                                )
                        nc.sync.dma_start(
                            keyT_dram[b, :, :, t * 512:(t + 1) * 512], kTt
                        )
                        for mc in range(KCK):
                            ps = kps.tile([128, 512], fp32, tag="mm")
                            for kc in range(KCK):
                                mm(ps, wk_sb[:, kc, mc * 128:(mc + 1) * 128],
                                   kTt[:, kc, :], kc == 0, kc == KCK - 1)
                            nc.vector.tensor_copy(
                                kT_sb[:, mc, t * 512:(t + 1) * 512], ps
                            )

                # ---- Phase B2: v = keyT^T Wv resident ----
                with (
                    tc.tile_pool(name="vproj", bufs=2) as v2p,
                    tc.tile_pool(name="wv", bufs=1) as wvp,
                    tc.tile_pool(name="vps", bufs=2, space="PSUM") as vps,
                ):
                    wv_sb = wvp.tile([128, KCK, DK], mdt)
                    nc.sync.dma_start(wv_sb, Wv.rearrange("(c p) n -> p c n", p=128))
                    for t in range(lk // 512):
                        kTt = v2p.tile([128, KCK, 512], mdt, tag="kTt2")
                        nc.sync.dma_start(
                            kTt, keyT_dram[b, :, :, t * 512:(t + 1) * 512]
                        )
                        for s in range(4):
                            for dk in range(2):
                                ps = vps.tile([128, 512], fp32, tag="vmm")
                                for kc in range(KCK):
                                    mm(ps, kTt[:, kc, s * 128:(s + 1) * 128],
                                       wv_sb[:, kc, dk * 512:(dk + 1) * 512],
                                       kc == 0, kc == KCK - 1)
                                nc.vector.tensor_copy(
                                    v_sb[:, t * 4 + s, dk * 512:(dk + 1) * 512], ps
                                )

                # ---- Phase C: attention ----
                with (
                    tc.tile_pool(name="attn", bufs=1) as cp,
                    tc.tile_pool(name="expp", bufs=NLK + 2) as ep,
                    tc.tile_pool(name="cps_s", bufs=2, space="PSUM") as cps_s,
                    tc.tile_pool(name="cps_o", bufs=2, space="PSUM") as cps_o,
                    tc.tile_pool(name="cps_n", bufs=2, space="PSUM") as cps_n,
                ):
                    for t in range(NCQ):
                        qTs = cp.tile([128, KCK, c_t], mdt, tag="qTs")
                        nc.sync.dma_start(
                            qTs, qT_dram[b, :, :, t * c_t:(t + 1) * c_t]
                        )
                        exps = []
                        for lkb in range(NLK):
                            ps_s = cps_s.tile([128, c_t], fp32, tag="sc")
                            for kc in range(KCK):
                                mm(ps_s, kT_sb[:, kc, lkb * 128:(lkb + 1) * 128],
                                   qTs[:, kc, :], kc == 0, kc == KCK - 1)
                            ex = ep.tile([128, c_t], mdt, tag="exp")
                            nc.scalar.activation(
                                ex, ps_s, mybir.ActivationFunctionType.Exp,
                                scale=1.0 / 32.0,
                            )
                            exps.append(ex)
                        for s in range(CS):
                            ps_o = cps_o.tile([128, DK], fp32, tag="pv")
                            ps_n = cps_n.tile([128, 4], fp32, tag="sum")
                            for lkb in range(NLK):
                                lhs = exps[lkb][:, s * 128:(s + 1) * 128]
                                for dk in range(2):
                                    mm(ps_o[:, dk * 512:(dk + 1) * 512], lhs,
                                       v_sb[:, lkb, dk * 512:(dk + 1) * 512],
                                       lkb == 0, lkb == NLK - 1)
                                mm(ps_n, lhs, ones_col, lkb == 0, lkb == NLK - 1)
                            rec = cp.tile([128, 1], fp32, tag="rec")
                            nc.vector.reciprocal(rec, ps_n[:, 0:1])
                            o_sb = cp.tile([128, DK], fp32, tag="osb")
                            nc.scalar.activation(
                                o_sb, ps_o,
                                mybir.ActivationFunctionType.Copy, scale=rec,
                            )
                            nc.vector.tensor_add(o_sb, o_sb, bv_rep)
                            nc.sync.dma_start(
                                out[b, t * c_t + s * 128: t * c_t + (s + 1) * 128, :],
                                o_sb,
                            )
    return nc


_NC_CACHE = {}


def _get_nc(key=("full",)):
    if key not in _NC_CACHE:
        _NC_CACHE[key] = build_nc()
    return _NC_CACHE[key]


def kernel(**inputs):
    from concourse.bass_utils import run_bass_kernel_spmd

    f32c = lambda x: np.ascontiguousarray(np.asarray(x), dtype=np.float32)
    query = f32c(inputs["query"])
    key = f32c(inputs["key"])
    shared = {n: f32c(inputs[n]) for n in ("Wq", "bq", "Wk", "Wv", "bv")}

    nc = _get_nc()
    if not nc.is_finalized():
        nc.finalize()
    in_maps = []
    for c in range(N_CORES):
        m = dict(shared)
        m["query"] = query[c * BPC:(c + 1) * BPC]
        m["key"] = key[c * BPC:(c + 1) * BPC]
        m["variant_tag"] = np.zeros((1, 8), np.float32)
        in_maps.append(m)

    res = run_bass_kernel_spmd(nc, in_maps, core_ids=list(range(N_CORES)))
    return np.concatenate([r["out"] for r in res.results], axis=0)



# revision 2
# speedup vs baseline: 1.4713x; 1.4713x over previous
"""CrossAttention Trainium2 Bass kernel (v2).

Problem (hardcoded): B=16, Lq=Lk=2048, Dq=768, Dk=1024, fp32.
  q = query @ Wq + bq ; k = key @ Wk + bk ; v = key @ Wv + bv
  out = softmax(q k^T / sqrt(1024)) @ v

Sharding: data-parallel over batch, 2 batches per core on 8 cores.

Math simplifications (exact up to rounding):
  - bk shifts every score row by a per-query constant -> cancels in softmax.
  - softmax weights sum to 1, so bv passes through attention unchanged:
    add bv once to the final output instead of to v.
  - scores are bounded (|s|/32 < ~3) so exp() without max-subtraction is safe.

v2 design (vs v1): query/key are transposed AND cast to bf16 on the HOST,
so the device does zero PE transposes; all matmul operands are bf16
(fp32 PSUM accumulate), which halves SBUF footprint and lets kT, v stay
fully SBUF-resident (no DRAM spills).  Phases per batch:
  B) kT = Wk^T keyT (resident), v = keyT^T Wv (resident), streamed over
     4 Lk tiles of 512.
  A+C fused per Lq tile of 512: qT tile = Wq^T queryT (+bq), then
     flash attention: scoresT = kT^T qT (8 dk chunks, PSUM), exp(s/32),
     out = sum_lk expT^T v with a ones-column matmul for row sums,
     normalize by reciprocal, + bv, DMA out (fp32).
"""

import numpy as np

B, LQ, LK = 16, 2048, 2048
DQ, DK = 768, 1024
N_CORES = 8
BPC = B // N_CORES  # batches per core

KCQ = DQ // 128  # 6 contraction chunks for q projection
KCK = DK // 128  # 8 contraction chunks for k/v projection + scores
NLK = LK // 128  # 16 Lk subtiles of 128


def build_nc(bpc=BPC, lq=LQ, lk=LK, c_t=512):
    import concourse.bass as bass
    import concourse.mybir as mybir
    from concourse import bacc
    import concourse.tile as tile

    fp32 = mybir.dt.float32
    bf16 = mybir.dt.bfloat16
    NCQ = lq // c_t  # Lq tiles (attention phase)
    CS = c_t // 128  # Lq subtiles per attention tile
    NTK = lk // 512  # Lk tiles (projection phase)

    nc = bacc.Bacc("TRN2")
    queryT = nc.dram_tensor("queryT", [bpc, DQ, lq], bf16, kind="ExternalInput")
    keyT = nc.dram_tensor("keyT", [bpc, DK, lk], bf16, kind="ExternalInput")
    Wq = nc.dram_tensor("Wq", [DQ, DK], bf16, kind="ExternalInput")
    bq = nc.dram_tensor("bq", [DK], fp32, kind="ExternalInput")
    Wk = nc.dram_tensor("Wk", [DK, DK], bf16, kind="ExternalInput")
    Wv = nc.dram_tensor("Wv", [DK, DK], bf16, kind="ExternalInput")
    bv = nc.dram_tensor("bv", [DK], fp32, kind="ExternalInput")
    out = nc.dram_tensor("out", [bpc, lq, DK], fp32, kind="ExternalOutput")

    def mm(ps, lhsT, rhs, start, stop):
        nc.tensor.matmul(ps, lhsT, rhs, start=start, stop=stop)

    with tile.TileContext(nc) as tc:
        with (
            tc.tile_pool(name="const", bufs=1) as constp,
            tc.tile_pool(name="w", bufs=1) as wp,
            tc.tile_pool(name="kT", bufs=1) as kTp,
            tc.tile_pool(name="v", bufs=1) as vp,
            tc.tile_pool(name="kstage", bufs=2) as ksp,
            tc.tile_pool(name="qstage", bufs=2) as qsp,
            tc.tile_pool(name="qT", bufs=2) as qTp,
            tc.tile_pool(name="exp", bufs=NLK + 2) as ep,
            tc.tile_pool(name="osb", bufs=2) as op,
            tc.tile_pool(name="ps512", bufs=2, space="PSUM") as ps512,
            tc.tile_pool(name="ps_o", bufs=2, space="PSUM") as ps_op,
            tc.tile_pool(name="ps_n", bufs=2, space="PSUM") as ps_np,
        ):
            ones_f32 = constp.tile([128, 4], fp32)
            nc.vector.memset(ones_f32, 1.0)
            ones_col = constp.tile([128, 4], bf16)
            nc.vector.tensor_copy(ones_col, ones_f32)
            bq_sb = constp.tile([128, KCK], fp32)
            nc.sync.dma_start(bq_sb, bq.rearrange("(c p) -> p c", p=128))
            bv_rep = constp.tile([128, DK], fp32)
            nc.sync.dma_start(bv_rep, bv[None, :].partition_broadcast(128))

            wq_sb = wp.tile([128, KCQ, DK], bf16)
            nc.sync.dma_start(wq_sb, Wq.rearrange("(c p) n -> p c n", p=128))
            wk_sb = wp.tile([128, KCK, DK], bf16)
            nc.sync.dma_start(wk_sb, Wk.rearrange("(c p) n -> p c n", p=128))
            wv_sb = wp.tile([128, KCK, DK], bf16)
            nc.sync.dma_start(wv_sb, Wv.rearrange("(c p) n -> p c n", p=128))

            for b in range(bpc):
                kT_sb = kTp.tile([128, KCK, lk], bf16, tag="kT")  # [dk, lk]
                v_sb = vp.tile([128, NLK, DK], bf16, tag="v")     # [lk, dk]

                # ---- Phase B: kT and v, SBUF resident ----
                for t in range(NTK):
                    kst = ksp.tile([128, KCK, 512], bf16, tag="kst")
                    nc.sync.dma_start(
                        kst,
                        keyT[b, :, t * 512:(t + 1) * 512].rearrange(
                            "(c p) n -> p c n", p=128
                        ),
                    )
                    for mc in range(KCK):
                        ps = ps512.tile([128, 512], fp32, tag="mm512")
                        for kc in range(KCK):
                            mm(ps, wk_sb[:, kc, mc * 128:(mc + 1) * 128],
                               kst[:, kc, :], kc == 0, kc == KCK - 1)
                        nc.vector.tensor_copy(
                            kT_sb[:, mc, t * 512:(t + 1) * 512], ps
                        )
                    for s in range(4):
                        for dk in range(2):
                            ps = ps_op.tile([128, DK], fp32, tag="pv")
                            for kc in range(KCK):
                                mm(ps[:, 0:512],
                                   kst[:, kc, s * 128:(s + 1) * 128],
                                   wv_sb[:, kc, dk * 512:(dk + 1) * 512],
                                   kc == 0, kc == KCK - 1)
                            nc.vector.tensor_copy(
                                v_sb[:, t * 4 + s, dk * 512:(dk + 1) * 512],
                                ps[:, 0:512],
                            )

                # ---- Phase A+C fused per Lq tile ----
                for t in range(NCQ):
                    qst = qsp.tile([128, KCQ, c_t], bf16, tag="qst")
                    nc.sync.dma_start(
                        qst,
                        queryT[b, :, t * c_t:(t + 1) * c_t].rearrange(
                            "(c p) n -> p c n", p=128
                        ),
                    )
                    qTt = qTp.tile([128, KCK, c_t], bf16, tag="qTt")
                    for mc in range(KCK):
                        ps = ps512.tile([128, c_t], fp32, tag="mm512")
                        for kc in range(KCQ):
                            mm(ps, wq_sb[:, kc, mc * 128:(mc + 1) * 128],
                               qst[:, kc, :], kc == 0, kc == KCQ - 1)
                        nc.vector.tensor_scalar_add(
                            qTt[:, mc, :], ps, bq_sb[:, mc:mc + 1]
                        )
                    exps = []
                    for lkb in range(NLK):
                        ps_s = ps512.tile([128, c_t], fp32, tag="mm512")
                        for kc in range(KCK):
                            mm(ps_s, kT_sb[:, kc, lkb * 128:(lkb + 1) * 128],
                               qTt[:, kc, :], kc == 0, kc == KCK - 1)
                        ex = ep.tile([128, c_t], bf16, tag="exp")
                        nc.scalar.activation(
                            ex, ps_s, mybir.ActivationFunctionType.Exp,
                            scale=1.0 / 32.0,
                        )
                        exps.append(ex)
                    for s in range(CS):
                        ps_o = ps_op.tile([128, DK], fp32, tag="pv")
                        ps_n = ps_np.tile([128, 4], fp32, tag="sum")
                        for lkb in range(NLK):
                            lhs = exps[lkb][:, s * 128:(s + 1) * 128]
                            for dk in range(2):
                                mm(ps_o[:, dk * 512:(dk + 1) * 512], lhs,
                                   v_sb[:, lkb, dk * 512:(dk + 1) * 512],
                                   lkb == 0, lkb == NLK - 1)
                            mm(ps_n, lhs, ones_col, lkb == 0, lkb == NLK - 1)
                        rec = op.tile([128, 1], fp32, tag="rec")
                        nc.vector.reciprocal(rec, ps_n[:, 0:1])
                        o_sb = op.tile([128, DK], fp32, tag="osb")
                        nc.scalar.activation(
                            o_sb, ps_o,
                            mybir.ActivationFunctionType.Copy, scale=rec,
                        )
                        nc.vector.tensor_add(o_sb, o_sb, bv_rep)
                        nc.sync.dma_start(
                            out[b, t * c_t + s * 128: t * c_t + (s + 1) * 128, :],
                            o_sb,
                        )
    return nc


_NC_CACHE = {}


def _get_nc(key=("v2",)):
    if key not in _NC_CACHE:
        _NC_CACHE[key] = build_nc()
    return _NC_CACHE[key]


def make_in_maps(inputs):
    """Host-side prep: cast to bf16, pre-transpose query/key, shard by batch."""
    import ml_dtypes

    bf16 = ml_dtypes.bfloat16
    f32c = lambda x: np.ascontiguousarray(np.asarray(x), dtype=np.float32)

    qT = np.ascontiguousarray(
        np.asarray(inputs["query"]).astype(bf16).transpose(0, 2, 1)
    )
    kT = np.ascontiguousarray(
        np.asarray(inputs["key"]).astype(bf16).transpose(0, 2, 1)
    )
    shared = {
        "Wq": np.asarray(inputs["Wq"]).astype(bf16),
        "Wk": np.asarray(inputs["Wk"]).astype(bf16),
        "Wv": np.asarray(inputs["Wv"]).astype(bf16),
        "bq": f32c(inputs["bq"]),
        "bv": f32c(inputs["bv"]),
    }
    in_maps = []
    for c in range(N_CORES):
        m = dict(shared)
        m["queryT"] = qT[c * BPC:(c + 1) * BPC]
        m["keyT"] = kT[c * BPC:(c + 1) * BPC]
        in_maps.append(m)
    return in_maps


def kernel(**inputs):
    from concourse.bass_utils import run_bass_kernel_spmd

    nc = _get_nc()
    if not nc.is_finalized():
        nc.finalize()
    in_maps = make_in_maps(inputs)
    res = run_bass_kernel_spmd(nc, in_maps, core_ids=list(range(N_CORES)))
    return np.concatenate([r["out"] for r in res.results], axis=0)


# revision 4
# speedup vs baseline: 1.5002x; 1.0197x over previous
"""CrossAttention Trainium2 Bass kernel (v3).

Problem (hardcoded): B=16, Lq=Lk=2048, Dq=768, Dk=1024, fp32.
  q = query @ Wq + bq ; k = key @ Wk + bk ; v = key @ Wv + bv
  out = softmax(q k^T / sqrt(1024)) @ v

Sharding: data-parallel over batch, 2 batches per core on 8 cores.

Math simplifications (exact up to rounding):
  - bk shifts every score row by a per-query constant -> cancels in softmax.
  - bv folds into v (softmax weights sum to 1): v' = k@Wv + bv gives
    out = (sum exp * v') / sum exp directly.
  - scores are bounded (|s|/32 < ~3) so exp() without max-subtraction is safe.

v3 (vs v2): all DMAs use host-prepacked SBUF-image layouts (128 fat
contiguous descriptors per transfer instead of ~1024 thin ones), cutting
DMA-issue serialization at kernel start; weight DMAs issue in phase order
(Wk, Wv first); output is downloaded bf16 and cast to fp32 on the host.
Device work per batch: kT = Wk^T keyT and v = keyT^T Wv + bv resident in
SBUF, then per 512-col Lq tile: qT = Wq^T queryT + bq, scoresT = kT^T qT,
exp(s/32), PV + ones-column row sums, out = PV * (1/sums).
"""

import numpy as np

B, LQ, LK = 16, 2048, 2048
DQ, DK = 768, 1024
N_CORES = 8
BPC = B // N_CORES  # batches per core

KCQ = DQ // 128  # 6 contraction chunks for q projection
KCK = DK // 128  # 8 contraction chunks for k/v projection + scores
NLK = LK // 128  # 16 Lk subtiles of 128
C_T = 512
NCQ = LQ // C_T  # Lq tiles
NTK = LK // 512  # Lk tiles (projection phase)


def build_nc(bpc=BPC, lq=LQ, lk=LK, c_t=C_T):
    import concourse.bass as bass
    import concourse.mybir as mybir
    from concourse import bacc
    import concourse.tile as tile

    fp32 = mybir.dt.float32
    bf16 = mybir.dt.bfloat16
    CS = c_t // 128  # Lq subtiles per attention tile

    nc = bacc.Bacc("TRN2")
    # Host-prepacked SBUF-image layouts: partition dim explicit, per-partition
    # slabs contiguous so every DMA is 128 fat descriptors.
    queryT = nc.dram_tensor(
        "queryT", [bpc, NCQ, 128, KCQ, c_t], bf16, kind="ExternalInput")
    keyT = nc.dram_tensor(
        "keyT", [bpc, NTK, 128, KCK, 512], bf16, kind="ExternalInput")
    Wq = nc.dram_tensor("Wq", [128, KCQ, DK], bf16, kind="ExternalInput")
    Wk = nc.dram_tensor("Wk", [128, KCK, DK], bf16, kind="ExternalInput")
    Wv = nc.dram_tensor("Wv", [128, KCK, DK], bf16, kind="ExternalInput")
    bq = nc.dram_tensor("bq", [DK], fp32, kind="ExternalInput")
    bv = nc.dram_tensor("bv", [DK], fp32, kind="ExternalInput")
    out = nc.dram_tensor("out", [bpc, lq, DK], bf16, kind="ExternalOutput")

    def mm(ps, lhsT, rhs, start, stop):
        nc.tensor.matmul(ps, lhsT, rhs, start=start, stop=stop)

    with tile.TileContext(nc) as tc:
        with (
            tc.tile_pool(name="const", bufs=1) as constp,
            tc.tile_pool(name="w", bufs=1) as wp,
            tc.tile_pool(name="kT", bufs=1) as kTp,
            tc.tile_pool(name="v", bufs=1) as vp,
            tc.tile_pool(name="kstage", bufs=2) as ksp,
            tc.tile_pool(name="qstage", bufs=2) as qsp,
            tc.tile_pool(name="qT", bufs=2) as qTp,
            tc.tile_pool(name="exp", bufs=NLK + 2) as ep,
            tc.tile_pool(name="osb", bufs=2) as op,
            tc.tile_pool(name="ps512", bufs=2, space="PSUM") as ps512,
            tc.tile_pool(name="ps_o", bufs=2, space="PSUM") as ps_op,
            tc.tile_pool(name="ps_n", bufs=2, space="PSUM") as ps_np,
        ):
            # Phase-order DMA issue: Wk halves first (first matmuls), then Wv
            # (needed ~10us in), bv (v evacuation), then Wq/bq (phase A).
            wk_sb = wp.tile([128, KCK, DK], bf16)
            nc.sync.dma_start(wk_sb[:, 0:4, :], Wk[:, 0:4, :])
            nc.sync.dma_start(wk_sb[:, 4:8, :], Wk[:, 4:8, :])
            wv_sb = wp.tile([128, KCK, DK], bf16)
            nc.sync.dma_start(wv_sb, Wv[:, :, :])
            bv_rep = constp.tile([128, DK], fp32)
            nc.sync.dma_start(bv_rep, bv[None, :].partition_broadcast(128))
            wq_sb = wp.tile([128, KCQ, DK], bf16)
            nc.sync.dma_start(wq_sb, Wq[:, :, :])
            bq_sb = constp.tile([128, KCK], fp32)
            nc.sync.dma_start(bq_sb, bq.rearrange("(c p) -> p c", p=128))
            ones_f32 = constp.tile([128, 4], fp32)
            nc.vector.memset(ones_f32, 1.0)
            ones_col = constp.tile([128, 4], bf16)
            nc.vector.tensor_copy(ones_col, ones_f32)

            for b in range(bpc):
                kT_sb = kTp.tile([128, KCK, lk], bf16, tag="kT")  # [dk, lk]
                v_sb = vp.tile([128, NLK, DK], bf16, tag="v")     # [lk, dk]

                # ---- Phase B: kT and v (+bv), SBUF resident ----
                for t in range(NTK):
                    kst = ksp.tile([128, KCK, 512], bf16, tag="kst")
                    nc.sync.dma_start(kst[:, 0:4, :], keyT[b, t, :, 0:4, :])
                    nc.sync.dma_start(kst[:, 4:8, :], keyT[b, t, :, 4:8, :])
                    for mc in range(KCK):
                        ps = ps512.tile([128, 512], fp32, tag="mm512")
                        for kc in range(KCK):
                            mm(ps, wk_sb[:, kc, mc * 128:(mc + 1) * 128],
                               kst[:, kc, :], kc == 0, kc == KCK - 1)
                        nc.vector.tensor_copy(
                            kT_sb[:, mc, t * 512:(t + 1) * 512], ps
                        )
                    for s in range(4):
                        for dk in range(2):
                            ps = ps_op.tile([128, DK], fp32, tag="pv")
                            for kc in range(KCK):
                                mm(ps[:, 0:512],
                                   kst[:, kc, s * 128:(s + 1) * 128],
                                   wv_sb[:, kc, dk * 512:(dk + 1) * 512],
                                   kc == 0, kc == KCK - 1)
                            nc.vector.tensor_add(
                                v_sb[:, t * 4 + s, dk * 512:(dk + 1) * 512],
                                ps[:, 0:512],
                                bv_rep[:, dk * 512:(dk + 1) * 512],
                            )

                # ---- Phase A+C fused per Lq tile ----
                for t in range(NCQ):
                    qst = qsp.tile([128, KCQ, c_t], bf16, tag="qst")
                    nc.sync.dma_start(qst, queryT[b, t])
                    qTt = qTp.tile([128, KCK, c_t], bf16, tag="qTt")
                    for mc in range(KCK):
                        ps = ps512.tile([128, c_t], fp32, tag="mm512")
                        for kc in range(KCQ):
                            mm(ps, wq_sb[:, kc, mc * 128:(mc + 1) * 128],
                               qst[:, kc, :], kc == 0, kc == KCQ - 1)
                        nc.vector.tensor_scalar_add(
                            qTt[:, mc, :], ps, bq_sb[:, mc:mc + 1]
                        )
                    exps = []
                    for lkb in range(NLK):
                        ps_s = ps512.tile([128, c_t], fp32, tag="mm512")
                        for kc in range(KCK):
                            mm(ps_s, kT_sb[:, kc, lkb * 128:(lkb + 1) * 128],
                               qTt[:, kc, :], kc == 0, kc == KCK - 1)
                        ex = ep.tile([128, c_t], bf16, tag="exp")
                        nc.scalar.activation(
                            ex, ps_s, mybir.ActivationFunctionType.Exp,
                            scale=1.0 / 32.0,
                        )
                        exps.append(ex)
                    for s in range(CS):
                        ps_o = ps_op.tile([128, DK], fp32, tag="pv")
                        ps_n = ps_np.tile([128, 4], fp32, tag="sum")
                        for lkb in range(NLK):
                            lhs = exps[lkb][:, s * 128:(s + 1) * 128]
                            for dk in range(2):
                                mm(ps_o[:, dk * 512:(dk + 1) * 512], lhs,
                                   v_sb[:, lkb, dk * 512:(dk + 1) * 512],
                                   lkb == 0, lkb == NLK - 1)
                            mm(ps_n, lhs, ones_col, lkb == 0, lkb == NLK - 1)
                        rec = op.tile([128, 1], fp32, tag="rec")
                        nc.vector.reciprocal(rec, ps_n[:, 0:1])
                        o_sb = op.tile([128, DK], bf16, tag="osb")
                        nc.scalar.activation(
                            o_sb, ps_o,
                            mybir.ActivationFunctionType.Copy, scale=rec,
                        )
                        nc.sync.dma_start(
                            out[b, t * c_t + s * 128: t * c_t + (s + 1) * 128, :],
                            o_sb,
                        )
    return nc


_NC_CACHE = {}


def _get_nc(key=("v3",)):
    if key not in _NC_CACHE:
        _NC_CACHE[key] = build_nc()
    return _NC_CACHE[key]


def make_in_maps(inputs):
    """Host prep: cast bf16, pack SBUF-image layouts, shard by batch."""
    import ml_dtypes

    bf16 = ml_dtypes.bfloat16
    f32c = lambda x: np.ascontiguousarray(np.asarray(x), dtype=np.float32)

    # query [B, lq, dq] -> [B, t, p, kc, n]: lq = t*512+n, dq = kc*128+p
    qT = np.ascontiguousarray(
        np.asarray(inputs["query"]).astype(bf16)
        .reshape(B, NCQ, C_T, KCQ, 128).transpose(0, 1, 4, 3, 2)
    )
    kT = np.ascontiguousarray(
        np.asarray(inputs["key"]).astype(bf16)
        .reshape(B, NTK, 512, KCK, 128).transpose(0, 1, 4, 3, 2)
    )
    # W [dk_in, dk_out] -> [p, kc, dk_out]: dk_in = kc*128+p
    def w_img(w, kc):
        return np.ascontiguousarray(
            np.asarray(w).astype(bf16).reshape(kc, 128, -1).transpose(1, 0, 2)
        )

    shared = {
        "Wq": w_img(inputs["Wq"], KCQ),
        "Wk": w_img(inputs["Wk"], KCK),
        "Wv": w_img(inputs["Wv"], KCK),
        "bq": f32c(inputs["bq"]),
        "bv": f32c(inputs["bv"]),
    }
    in_maps = []
    for c in range(N_CORES):
        m = dict(shared)
        m["queryT"] = qT[c * BPC:(c + 1) * BPC]
        m["keyT"] = kT[c * BPC:(c + 1) * BPC]
        in_maps.append(m)
    return in_maps


def kernel(**inputs):
    from concourse.bass_utils import run_bass_kernel_spmd

    nc = _get_nc()
    if not nc.is_finalized():
        nc.finalize()
    in_maps = make_in_maps(inputs)
    res = run_bass_kernel_spmd(nc, in_maps, core_ids=list(range(N_CORES)))
    return np.concatenate(
        [r["out"].astype(np.float32) for r in res.results], axis=0
    )


# revision 6
# speedup vs baseline: 1.5143x; 1.0094x over previous
"""CrossAttention Trainium2 Bass kernel (v3).

Problem (hardcoded): B=16, Lq=Lk=2048, Dq=768, Dk=1024, fp32.
  q = query @ Wq + bq ; k = key @ Wk + bk ; v = key @ Wv + bv
  out = softmax(q k^T / sqrt(1024)) @ v

Sharding: data-parallel over batch, 2 batches per core on 8 cores.

Math simplifications (exact up to rounding):
  - bk shifts every score row by a per-query constant -> cancels in softmax.
  - bv folds into v (softmax weights sum to 1): v' = k@Wv + bv gives
    out = (sum exp * v') / sum exp directly.
  - scores are bounded (|s|/32 < ~3) so exp() without max-subtraction is safe.

v3 (vs v2): all DMAs use host-prepacked SBUF-image layouts (128 fat
contiguous descriptors per transfer instead of ~1024 thin ones), cutting
DMA-issue serialization at kernel start; weight DMAs issue in phase order
(Wk, Wv first); output is downloaded bf16 and cast to fp32 on the host.
Device work per batch: kT = Wk^T keyT and v = keyT^T Wv + bv resident in
SBUF, then per 512-col Lq tile: qT = Wq^T queryT + bq, scoresT = kT^T qT,
exp(s/32), PV + ones-column row sums, out = PV * (1/sums).
"""

import numpy as np

B, LQ, LK = 16, 2048, 2048
DQ, DK = 768, 1024
N_CORES = 8
BPC = B // N_CORES  # batches per core

KCQ = DQ // 128  # 6 contraction chunks for q projection
KCK = DK // 128  # 8 contraction chunks for k/v projection + scores
NLK = LK // 128  # 16 Lk subtiles of 128
C_T = 512
NCQ = LQ // C_T  # Lq tiles
NTK = LK // 512  # Lk tiles (projection phase)


def build_nc(bpc=BPC, lq=LQ, lk=LK, c_t=C_T):
    import concourse.bass as bass
    import concourse.mybir as mybir
    from concourse import bacc
    import concourse.tile as tile

    fp32 = mybir.dt.float32
    bf16 = mybir.dt.bfloat16
    CS = c_t // 128  # Lq subtiles per attention tile

    nc = bacc.Bacc("TRN2")
    # Host-prepacked SBUF-image layouts: partition dim explicit, per-partition
    # slabs contiguous so every DMA is 128 fat descriptors.
    queryT = nc.dram_tensor(
        "queryT", [bpc, NCQ, 128, KCQ, c_t], bf16, kind="ExternalInput")
    keyT = nc.dram_tensor(
        "keyT", [bpc, NTK, 128, KCK, 512], bf16, kind="ExternalInput")
    Wq = nc.dram_tensor("Wq", [128, KCQ, DK], bf16, kind="ExternalInput")
    Wk = nc.dram_tensor("Wk", [128, KCK, DK], bf16, kind="ExternalInput")
    Wv = nc.dram_tensor("Wv", [128, KCK, DK], bf16, kind="ExternalInput")
    bq = nc.dram_tensor("bq", [DK], fp32, kind="ExternalInput")
    bv = nc.dram_tensor("bv", [DK], fp32, kind="ExternalInput")
    out = nc.dram_tensor("out", [bpc, lq, DK], bf16, kind="ExternalOutput")

    def mm(ps, lhsT, rhs, start, stop):
        nc.tensor.matmul(ps, lhsT, rhs, start=start, stop=stop)

    with tile.TileContext(nc) as tc:
        with (
            tc.tile_pool(name="const", bufs=1) as constp,
            tc.tile_pool(name="w", bufs=1) as wp,
            tc.tile_pool(name="kT", bufs=1) as kTp,
            tc.tile_pool(name="v", bufs=1) as vp,
            tc.tile_pool(name="kstage", bufs=2) as ksp,
            tc.tile_pool(name="qstage", bufs=2) as qsp,
            tc.tile_pool(name="qT", bufs=2) as qTp,
            tc.tile_pool(name="exp", bufs=NLK + 2) as ep,
            tc.tile_pool(name="osb", bufs=2) as op,
            tc.tile_pool(name="ps512", bufs=2, space="PSUM") as ps512,
            tc.tile_pool(name="ps_o", bufs=2, space="PSUM") as ps_op,
            tc.tile_pool(name="ps_n", bufs=2, space="PSUM") as ps_np,
        ):
            # Phase-order DMA issue: Wk halves first (first matmuls), then Wv
            # (needed ~10us in), bv (v evacuation), then Wq/bq (phase A).
            wk_sb = wp.tile([128, KCK, DK], bf16)
            nc.sync.dma_start(wk_sb[:, 0:4, :], Wk[:, 0:4, :])
            nc.sync.dma_start(wk_sb[:, 4:8, :], Wk[:, 4:8, :])
            # First key tile right behind Wk so the first matmul can start
            # ~12us in instead of queueing behind all the other weights.
            kst0 = ksp.tile([128, KCK, 512], bf16, tag="kst")
            nc.sync.dma_start(kst0[:, 0:4, :], keyT[0, 0, :, 0:4, :])
            nc.sync.dma_start(kst0[:, 4:8, :], keyT[0, 0, :, 4:8, :])
            wv_sb = wp.tile([128, KCK, DK], bf16)
            nc.sync.dma_start(wv_sb, Wv[:, :, :])
            bv_rep = constp.tile([128, DK], fp32)
            nc.sync.dma_start(bv_rep, bv[None, :].partition_broadcast(128))
            wq_sb = wp.tile([128, KCQ, DK], bf16)
            nc.sync.dma_start(wq_sb, Wq[:, :, :])
            bq_sb = constp.tile([128, KCK], fp32)
            nc.sync.dma_start(bq_sb, bq.rearrange("(c p) -> p c", p=128))
            ones_f32 = constp.tile([128, 4], fp32)
            nc.vector.memset(ones_f32, 1.0)
            ones_col = constp.tile([128, 4], bf16)
            nc.vector.tensor_copy(ones_col, ones_f32)

            for b in range(bpc):
                kT_sb = kTp.tile([128, KCK, lk], bf16, tag="kT")  # [dk, lk]
                v_sb = vp.tile([128, NLK, DK], bf16, tag="v")     # [lk, dk]

                # ---- Phase B: kT and v (+bv), SBUF resident ----
                for t in range(NTK):
                    if b == 0 and t == 0:
                        kst = kst0
                    else:
                        kst = ksp.tile([128, KCK, 512], bf16, tag="kst")
                        nc.sync.dma_start(kst[:, 0:4, :], keyT[b, t, :, 0:4, :])
                        nc.sync.dma_start(kst[:, 4:8, :], keyT[b, t, :, 4:8, :])
                    for mc in range(KCK):
                        ps = ps512.tile([128, 512], fp32, tag="mm512")
                        for kc in range(KCK):
                            mm(ps, wk_sb[:, kc, mc * 128:(mc + 1) * 128],
                               kst[:, kc, :], kc == 0, kc == KCK - 1)
                        nc.vector.tensor_copy(
                            kT_sb[:, mc, t * 512:(t + 1) * 512], ps
                        )
                    for s in range(4):
                        for dk in range(2):
                            ps = ps_op.tile([128, DK], fp32, tag="pv")
                            for kc in range(KCK):
                                mm(ps[:, 0:512],
                                   kst[:, kc, s * 128:(s + 1) * 128],
                                   wv_sb[:, kc, dk * 512:(dk + 1) * 512],
                                   kc == 0, kc == KCK - 1)
                            nc.vector.tensor_add(
                                v_sb[:, t * 4 + s, dk * 512:(dk + 1) * 512],
                                ps[:, 0:512],
                                bv_rep[:, dk * 512:(dk + 1) * 512],
                            )

                # ---- Phase A+C fused per Lq tile ----
                for t in range(NCQ):
                    qst = qsp.tile([128, KCQ, c_t], bf16, tag="qst")
                    nc.sync.dma_start(qst, queryT[b, t])
                    qTt = qTp.tile([128, KCK, c_t], bf16, tag="qTt")
                    for mc in range(KCK):
                        ps = ps512.tile([128, c_t], fp32, tag="mm512")
                        for kc in range(KCQ):
                            mm(ps, wq_sb[:, kc, mc * 128:(mc + 1) * 128],
                               qst[:, kc, :], kc == 0, kc == KCQ - 1)
                        nc.vector.tensor_scalar_add(
                            qTt[:, mc, :], ps, bq_sb[:, mc:mc + 1]
                        )
                    exps = []
                    for lkb in range(NLK):
                        ps_s = ps512.tile([128, c_t], fp32, tag="mm512")
                        for kc in range(KCK):
                            mm(ps_s, kT_sb[:, kc, lkb * 128:(lkb + 1) * 128],
                               qTt[:, kc, :], kc == 0, kc == KCK - 1)
                        ex = ep.tile([128, c_t], bf16, tag="exp")
                        nc.scalar.activation(
                            ex, ps_s, mybir.ActivationFunctionType.Exp,
                            scale=1.0 / 32.0,
                        )
                        exps.append(ex)
                    for s in range(CS):
                        ps_o = ps_op.tile([128, DK], fp32, tag="pv")
                        ps_n = ps_np.tile([128, 4], fp32, tag="sum")
                        for lkb in range(NLK):
                            lhs = exps[lkb][:, s * 128:(s + 1) * 128]
                            for dk in range(2):
                                mm(ps_o[:, dk * 512:(dk + 1) * 512], lhs,
                                   v_sb[:, lkb, dk * 512:(dk + 1) * 512],
                                   lkb == 0, lkb == NLK - 1)
                            mm(ps_n, lhs, ones_col, lkb == 0, lkb == NLK - 1)
                        rec = op.tile([128, 1], fp32, tag="rec")
                        nc.vector.reciprocal(rec, ps_n[:, 0:1])
                        o_sb = op.tile([128, DK], bf16, tag="osb")
                        nc.scalar.activation(
                            o_sb, ps_o,
                            mybir.ActivationFunctionType.Copy, scale=rec,
                        )
                        nc.sync.dma_start(
                            out[b, t * c_t + s * 128: t * c_t + (s + 1) * 128, :],
                            o_sb,
                        )
    return nc


_NC_CACHE = {}


def _get_nc(key=("v3",)):
    if key not in _NC_CACHE:
        _NC_CACHE[key] = build_nc()
    return _NC_CACHE[key]


def make_in_maps(inputs):
    """Host prep: cast bf16, pack SBUF-image layouts, shard by batch."""
    import ml_dtypes

    bf16 = ml_dtypes.bfloat16
    f32c = lambda x: np.ascontiguousarray(np.asarray(x), dtype=np.float32)

    # query [B, lq, dq] -> [B, t, p, kc, n]: lq = t*512+n, dq = kc*128+p
    qT = np.ascontiguousarray(
        np.asarray(inputs["query"]).astype(bf16)
        .reshape(B, NCQ, C_T, KCQ, 128).transpose(0, 1, 4, 3, 2)
    )
    kT = np.ascontiguousarray(
        np.asarray(inputs["key"]).astype(bf16)
        .reshape(B, NTK, 512, KCK, 128).transpose(0, 1, 4, 3, 2)
    )
    # W [dk_in, dk_out] -> [p, kc, dk_out]: dk_in = kc*128+p
    def w_img(w, kc):
        return np.ascontiguousarray(
            np.asarray(w).astype(bf16).reshape(kc, 128, -1).transpose(1, 0, 2)
        )

    shared = {
        "Wq": w_img(inputs["Wq"], KCQ),
        "Wk": w_img(inputs["Wk"], KCK),
        "Wv": w_img(inputs["Wv"], KCK),
        "bq": f32c(inputs["bq"]),
        "bv": f32c(inputs["bv"]),
    }
    in_maps = []
    for c in range(N_CORES):
        m = dict(shared)
        m["queryT"] = qT[c * BPC:(c + 1) * BPC]
        m["keyT"] = kT[c * BPC:(c + 1) * BPC]
        in_maps.append(m)
    return in_maps


def kernel(**inputs):
    from concourse.bass_utils import run_bass_kernel_spmd

    nc = _get_nc()
    if not nc.is_finalized():
        nc.finalize()
    in_maps = make_in_maps(inputs)
    res = run_bass_kernel_spmd(nc, in_maps, core_ids=list(range(N_CORES)))
    return np.concatenate(
        [r["out"].astype(np.float32) for r in res.results], axis=0
    )


# revision 7
# speedup vs baseline: 1.5152x; 1.0005x over previous
"""CrossAttention Trainium2 Bass kernel (v4).

Problem (hardcoded): B=16, Lq=Lk=2048, Dq=768, Dk=1024, fp32.
  q = query @ Wq + bq ; k = key @ Wk + bk ; v = key @ Wv + bv
  out = softmax(q k^T / sqrt(1024)) @ v

Sharding: data-parallel over batch, 2 batches per core on 8 cores.

Math simplifications (exact up to rounding):
  - bk shifts every score row by a per-query constant -> cancels in softmax.
  - bv folds into v (softmax weights sum to 1): v' = k@Wv + bv gives
    out = (sum exp * v') / sum exp directly.
  - scores are bounded (|s|/32 < ~3) so exp() without max-subtraction is safe.

v3 (vs v2): all DMAs use host-prepacked SBUF-image layouts (128 fat
contiguous descriptors per transfer instead of ~1024 thin ones), cutting
DMA-issue serialization at kernel start; weight DMAs issue in phase order
(Wk, Wv first); output is downloaded bf16 and cast to fp32 on the host.
Device work per batch: kT = Wk^T keyT and v = keyT^T Wv + bv resident in
SBUF, then per 512-col Lq tile: qT = Wq^T queryT + bq, scoresT = kT^T qT,
exp(s/32), PV + ones-column row sums, out = PV * (1/sums).
"""

import numpy as np

B, LQ, LK = 16, 2048, 2048
DQ, DK = 768, 1024
N_CORES = 8
BPC = B // N_CORES  # batches per core

KCQ = DQ // 128  # 6 contraction chunks for q projection
KCK = DK // 128  # 8 contraction chunks for k/v projection + scores
NLK = LK // 128  # 16 Lk subtiles of 128
C_T = 512
NCQ = LQ // C_T  # Lq tiles
NTK = LK // 512  # Lk tiles (projection phase)


def build_nc(bpc=BPC, lq=LQ, lk=LK, c_t=C_T):
    import concourse.bass as bass
    import concourse.mybir as mybir
    from concourse import bacc
    import concourse.tile as tile

    fp32 = mybir.dt.float32
    bf16 = mybir.dt.bfloat16
    CS = c_t // 128  # Lq subtiles per attention tile

    nc = bacc.Bacc("TRN2")
    # Host-prepacked SBUF-image layouts: partition dim explicit, per-partition
    # slabs contiguous so every DMA is 128 fat descriptors.
    queryT = nc.dram_tensor(
        "queryT", [bpc, NCQ, 128, KCQ, c_t], bf16, kind="ExternalInput")
    keyT = nc.dram_tensor(
        "keyT", [bpc, NTK, 128, KCK, 512], bf16, kind="ExternalInput")
    Wq = nc.dram_tensor("Wq", [128, KCQ, DK], bf16, kind="ExternalInput")
    Wk = nc.dram_tensor("Wk", [128, KCK, DK], bf16, kind="ExternalInput")
    Wv = nc.dram_tensor("Wv", [128, KCK, DK], bf16, kind="ExternalInput")
    bq = nc.dram_tensor("bq", [DK], fp32, kind="ExternalInput")
    bv = nc.dram_tensor("bv", [DK], fp32, kind="ExternalInput")
    out = nc.dram_tensor("out", [bpc, lq, DK], bf16, kind="ExternalOutput")

    def mm(ps, lhsT, rhs, start, stop):
        nc.tensor.matmul(ps, lhsT, rhs, start=start, stop=stop)

    with tile.TileContext(nc) as tc:
        with (
            tc.tile_pool(name="const", bufs=1) as constp,
            tc.tile_pool(name="w", bufs=1) as wp,
            tc.tile_pool(name="kT", bufs=1) as kTp,
            tc.tile_pool(name="v", bufs=1) as vp,
            tc.tile_pool(name="kstage", bufs=2) as ksp,
            tc.tile_pool(name="qstage", bufs=2) as qsp,
            tc.tile_pool(name="qT", bufs=2) as qTp,
            tc.tile_pool(name="exp", bufs=NLK + 2) as ep,
            tc.tile_pool(name="osb", bufs=2) as op,
            tc.tile_pool(name="ps512", bufs=2, space="PSUM") as ps512,
            tc.tile_pool(name="ps_o", bufs=2, space="PSUM") as ps_op,
            tc.tile_pool(name="ps_n", bufs=2, space="PSUM") as ps_np,
        ):
            # Phase-order DMA issue: Wk halves first (first matmuls), then Wv
            # (needed ~10us in), bv (v evacuation), then Wq/bq (phase A).
            wk_sb = wp.tile([128, KCK, DK], bf16)
            nc.sync.dma_start(wk_sb[:, 0:4, :], Wk[:, 0:4, :])
            nc.sync.dma_start(wk_sb[:, 4:8, :], Wk[:, 4:8, :])
            # First key tile right behind Wk so the first matmul can start
            # ~12us in instead of queueing behind all the other weights.
            kst0 = ksp.tile([128, KCK, 512], bf16, tag="kst")
            nc.sync.dma_start(kst0[:, 0:4, :], keyT[0, 0, :, 0:4, :])
            nc.sync.dma_start(kst0[:, 4:8, :], keyT[0, 0, :, 4:8, :])
            wv_sb = wp.tile([128, KCK, DK], bf16)
            nc.sync.dma_start(wv_sb, Wv[:, :, :])
            bv_rep = constp.tile([128, DK], fp32)
            nc.sync.dma_start(bv_rep, bv[None, :].partition_broadcast(128))
            wq_sb = wp.tile([128, KCQ, DK], bf16)
            nc.sync.dma_start(wq_sb, Wq[:, :, :])
            bq_sb = constp.tile([128, KCK], fp32)
            nc.sync.dma_start(bq_sb, bq.rearrange("(c p) -> p c", p=128))
            ones_f32 = constp.tile([128, 4], fp32)
            nc.vector.memset(ones_f32, 1.0)
            ones_col = constp.tile([128, 4], bf16)
            nc.vector.tensor_copy(ones_col, ones_f32)

            for b in range(bpc):
                kT_sb = kTp.tile([128, KCK, lk], bf16, tag="kT")  # [dk, lk]
                v_sb = vp.tile([128, NLK, DK], bf16, tag="v")     # [lk, dk]

                # ---- Phase B: kT and v (+bv), SBUF resident ----
                for t in range(NTK):
                    if b == 0 and t == 0:
                        kst = kst0
                    else:
                        kst = ksp.tile([128, KCK, 512], bf16, tag="kst")
                        nc.sync.dma_start(kst[:, 0:4, :], keyT[b, t, :, 0:4, :])
                        nc.sync.dma_start(kst[:, 4:8, :], keyT[b, t, :, 4:8, :])
                    for mc in range(KCK):
                        ps = ps512.tile([128, 512], fp32, tag="mm512")
                        for kc in range(KCK):
                            mm(ps, wk_sb[:, kc, mc * 128:(mc + 1) * 128],
                               kst[:, kc, :], kc == 0, kc == KCK - 1)
                        nc.vector.tensor_copy(
                            kT_sb[:, mc, t * 512:(t + 1) * 512], ps
                        )
                    for s in range(4):
                        for dk in range(2):
                            ps = ps_op.tile([128, DK], fp32, tag="pv")
                            for kc in range(KCK):
                                mm(ps[:, 0:512],
                                   kst[:, kc, s * 128:(s + 1) * 128],
                                   wv_sb[:, kc, dk * 512:(dk + 1) * 512],
                                   kc == 0, kc == KCK - 1)
                            nc.vector.tensor_add(
                                v_sb[:, t * 4 + s, dk * 512:(dk + 1) * 512],
                                ps[:, 0:512],
                                bv_rep[:, dk * 512:(dk + 1) * 512],
                            )

                # ---- Phase A+C fused per Lq tile ----
                for t in range(NCQ):
                    qst = qsp.tile([128, KCQ, c_t], bf16, tag="qst")
                    nc.sync.dma_start(qst, queryT[b, t])
                    qTt = qTp.tile([128, KCK, c_t], bf16, tag="qTt")
                    for mc in range(KCK):
                        ps = ps512.tile([128, c_t], fp32, tag="mm512")
                        for kc in range(KCQ):
                            mm(ps, wq_sb[:, kc, mc * 128:(mc + 1) * 128],
                               qst[:, kc, :], kc == 0, kc == KCQ - 1)
                        nc.vector.tensor_scalar_add(
                            qTt[:, mc, :], ps, bq_sb[:, mc:mc + 1]
                        )
                    exps = []
                    for lkb in range(NLK):
                        ps_s = ps512.tile([128, c_t], fp32, tag="mm512")
                        for kc in range(KCK):
                            mm(ps_s, kT_sb[:, kc, lkb * 128:(lkb + 1) * 128],
                               qTt[:, kc, :], kc == 0, kc == KCK - 1)
                        ex = ep.tile([128, c_t], bf16, tag="exp")
                        nc.scalar.activation(
                            ex, ps_s, mybir.ActivationFunctionType.Exp,
                            scale=1.0 / 32.0,
                        )
                        exps.append(ex)
                    for s in range(CS):
                        ps_o = ps_op.tile([128, DK], fp32, tag="pv")
                        ps_n = ps_np.tile([128, 4], fp32, tag="sum")
                        for lkb in range(NLK):
                            lhs = exps[lkb][:, s * 128:(s + 1) * 128]
                            for dk in range(2):
                                mm(ps_o[:, dk * 512:(dk + 1) * 512], lhs,
                                   v_sb[:, lkb, dk * 512:(dk + 1) * 512],
                                   lkb == 0, lkb == NLK - 1)
                            mm(ps_n, lhs, ones_col, lkb == 0, lkb == NLK - 1)
                        rec = op.tile([128, 1], fp32, tag="rec")
                        nc.vector.reciprocal(rec, ps_n[:, 0:1])
                        o_sb = op.tile([128, DK], bf16, tag="osb")
                        nc.scalar.activation(
                            o_sb, ps_o,
                            mybir.ActivationFunctionType.Copy, scale=rec,
                        )
                        nc.sync.dma_start(
                            out[b, t * c_t + s * 128: t * c_t + (s + 1) * 128, :],
                            o_sb,
                        )
    return nc


_NC_CACHE = {}


def _get_nc(key=("v3",)):
    if key not in _NC_CACHE:
        _NC_CACHE[key] = build_nc()
    return _NC_CACHE[key]


def make_in_maps(inputs):
    """Host prep: cast bf16, pack SBUF-image layouts, shard by batch."""
    import ml_dtypes

    bf16 = ml_dtypes.bfloat16
    f32c = lambda x: np.ascontiguousarray(np.asarray(x), dtype=np.float32)

    # query [B, lq, dq] -> [B, t, p, kc, n]: lq = t*512+n, dq = kc*128+p
    qT = np.ascontiguousarray(
        np.asarray(inputs["query"]).astype(bf16)
        .reshape(B, NCQ, C_T, KCQ, 128).transpose(0, 1, 4, 3, 2)
    )
    kT = np.ascontiguousarray(
        np.asarray(inputs["key"]).astype(bf16)
        .reshape(B, NTK, 512, KCK, 128).transpose(0, 1, 4, 3, 2)
    )
    # W [dk_in, dk_out] -> [p, kc, dk_out]: dk_in = kc*128+p
    def w_img(w, kc):
        return np.ascontiguousarray(
            np.asarray(w).astype(bf16).reshape(kc, 128, -1).transpose(1, 0, 2)
        )

    shared = {
        "Wq": w_img(inputs["Wq"], KCQ),
        "Wk": w_img(inputs["Wk"], KCK),
        "Wv": w_img(inputs["Wv"], KCK),
        "bq": f32c(inputs["bq"]),
        "bv": f32c(inputs["bv"]),
    }
    in_maps = []
    for c in range(N_CORES):
        m = dict(shared)
        m["queryT"] = qT[c * BPC:(c + 1) * BPC]
        m["keyT"] = kT[c * BPC:(c + 1) * BPC]
        in_maps.append(m)
    return in_maps


def kernel(**inputs):
    from concourse.bass_utils import run_bass_kernel_spmd

    nc = _get_nc()
    if not nc.is_finalized():
        nc.finalize()
    in_maps = make_in_maps(inputs)
    res = run_bass_kernel_spmd(nc, in_maps, core_ids=list(range(N_CORES)))
    return np.concatenate(
        [r["out"].astype(np.float32) for r in res.results], axis=0
    )


# revision 8
# speedup vs baseline: 1.5228x; 1.0051x over previous
"""CrossAttention Trainium2 Bass kernel (v4).

Problem (hardcoded): B=16, Lq=Lk=2048, Dq=768, Dk=1024, fp32.
  q = query @ Wq + bq ; k = key @ Wk + bk ; v = key @ Wv + bv
  out = softmax(q k^T / sqrt(1024)) @ v

Sharding: data-parallel over batch, 2 batches per core on 8 cores.

Math simplifications (exact up to rounding):
  - bk shifts every score row by a per-query constant -> cancels in softmax.
  - bv folds into v (softmax weights sum to 1): v' = k@Wv + bv gives
    out = (sum exp * v') / sum exp directly.
  - scores are bounded (|s|/32 < ~3) so exp() without max-subtraction is safe.

v3 (vs v2): all DMAs use host-prepacked SBUF-image layouts (128 fat
contiguous descriptors per transfer instead of ~1024 thin ones), cutting
DMA-issue serialization at kernel start; weight DMAs issue in phase order
(Wk, Wv first); output is downloaded bf16 and cast to fp32 on the host.
Device work per batch: kT = Wk^T keyT and v = keyT^T Wv + bv resident in
SBUF, then per 512-col Lq tile: qT = Wq^T queryT + bq, scoresT = kT^T qT,
exp(s/32), PV + ones-column row sums, out = PV * (1/sums).
"""

import numpy as np

B, LQ, LK = 16, 2048, 2048
DQ, DK = 768, 1024
N_CORES = 8
BPC = B // N_CORES  # batches per core

KCQ = DQ // 128  # 6 contraction chunks for q projection
KCK = DK // 128  # 8 contraction chunks for k/v projection + scores
NLK = LK // 128  # 16 Lk subtiles of 128
C_T = 512
NCQ = LQ // C_T  # Lq tiles
NTK = LK // 512  # Lk tiles (projection phase)


def build_nc(bpc=BPC, lq=LQ, lk=LK, c_t=C_T):
    import concourse.bass as bass
    import concourse.mybir as mybir
    from concourse import bacc
    import concourse.tile as tile

    fp32 = mybir.dt.float32
    bf16 = mybir.dt.bfloat16
    CS = c_t // 128  # Lq subtiles per attention tile

    nc = bacc.Bacc("TRN2")
    # Host-prepacked SBUF-image layouts: partition dim explicit, per-partition
    # slabs contiguous so every DMA is 128 fat descriptors.
    queryT = nc.dram_tensor(
        "queryT", [bpc, NCQ, 128, KCQ, c_t], bf16, kind="ExternalInput")
    keyT = nc.dram_tensor(
        "keyT", [bpc, NTK, 128, KCK, 512], bf16, kind="ExternalInput")
    Wq = nc.dram_tensor("Wq", [128, KCQ, DK], bf16, kind="ExternalInput")
    Wk = nc.dram_tensor("Wk", [128, KCK, DK], bf16, kind="ExternalInput")
    Wv = nc.dram_tensor("Wv", [128, KCK, DK], bf16, kind="ExternalInput")
    bq = nc.dram_tensor("bq", [DK], fp32, kind="ExternalInput")
    bv = nc.dram_tensor("bv", [DK], fp32, kind="ExternalInput")
    out = nc.dram_tensor("out", [bpc, lq, DK], bf16, kind="ExternalOutput")

    def mm(ps, lhsT, rhs, start, stop):
        nc.tensor.matmul(ps, lhsT, rhs, start=start, stop=stop)

    with tile.TileContext(nc) as tc:
        with (
            tc.tile_pool(name="const", bufs=1) as constp,
            tc.tile_pool(name="w", bufs=1) as wp,
            tc.tile_pool(name="kT", bufs=1) as kTp,
            tc.tile_pool(name="v", bufs=1) as vp,
            tc.tile_pool(name="kstage", bufs=2) as ksp,
            tc.tile_pool(name="qstage", bufs=2) as qsp,
            tc.tile_pool(name="qT", bufs=2) as qTp,
            tc.tile_pool(name="exp", bufs=NLK + 2) as ep,
            tc.tile_pool(name="osb", bufs=2) as op,
            tc.tile_pool(name="ps512", bufs=2, space="PSUM") as ps512,
            tc.tile_pool(name="ps_o", bufs=2, space="PSUM") as ps_op,
            tc.tile_pool(name="ps_n", bufs=2, space="PSUM") as ps_np,
        ):
            # Phase-order DMA issue: Wk halves first (first matmuls), then Wv
            # (needed ~10us in), bv (v evacuation), then Wq/bq (phase A).
            # Interleave Wk and the first key tile so the first accumulation
            # group's operands (kc 0-3 of both) land earliest; the kc 4-7
            # halves stream in during the first matmul group.
            wk_sb = wp.tile([128, KCK, DK], bf16)
            kst0 = ksp.tile([128, KCK, 512], bf16, tag="kst")
            nc.sync.dma_start(wk_sb[:, 0:4, :], Wk[:, 0:4, :])
            nc.sync.dma_start(kst0[:, 0:4, :], keyT[0, 0, :, 0:4, :])
            nc.sync.dma_start(wk_sb[:, 4:8, :], Wk[:, 4:8, :])
            nc.sync.dma_start(kst0[:, 4:8, :], keyT[0, 0, :, 4:8, :])
            wv_sb = wp.tile([128, KCK, DK], bf16)
            nc.sync.dma_start(wv_sb, Wv[:, :, :])
            bv_rep = constp.tile([128, DK], fp32)
            nc.sync.dma_start(bv_rep, bv[None, :].partition_broadcast(128))
            wq_sb = wp.tile([128, KCQ, DK], bf16)
            nc.sync.dma_start(wq_sb, Wq[:, :, :])
            bq_sb = constp.tile([128, KCK], fp32)
            nc.sync.dma_start(bq_sb, bq.rearrange("(c p) -> p c", p=128))
            ones_f32 = constp.tile([128, 4], fp32)
            nc.vector.memset(ones_f32, 1.0)
            ones_col = constp.tile([128, 4], bf16)
            nc.vector.tensor_copy(ones_col, ones_f32)

            for b in range(bpc):
                kT_sb = kTp.tile([128, KCK, lk], bf16, tag="kT")  # [dk, lk]
                v_sb = vp.tile([128, NLK, DK], bf16, tag="v")     # [lk, dk]

                # ---- Phase B: kT and v (+bv), SBUF resident ----
                for t in range(NTK):
                    if b == 0 and t == 0:
                        kst = kst0
                    else:
                        kst = ksp.tile([128, KCK, 512], bf16, tag="kst")
                        nc.sync.dma_start(kst[:, 0:4, :], keyT[b, t, :, 0:4, :])
                        nc.sync.dma_start(kst[:, 4:8, :], keyT[b, t, :, 4:8, :])
                    for mc in range(KCK):
                        ps = ps512.tile([128, 512], fp32, tag="mm512")
                        for kc in range(KCK):
                            mm(ps, wk_sb[:, kc, mc * 128:(mc + 1) * 128],
                               kst[:, kc, :], kc == 0, kc == KCK - 1)
                        nc.vector.tensor_copy(
                            kT_sb[:, mc, t * 512:(t + 1) * 512], ps
                        )
                    for s in range(4):
                        for dk in range(2):
                            ps = ps_op.tile([128, DK], fp32, tag="pv")
                            for kc in range(KCK):
                                mm(ps[:, 0:512],
                                   kst[:, kc, s * 128:(s + 1) * 128],
                                   wv_sb[:, kc, dk * 512:(dk + 1) * 512],
                                   kc == 0, kc == KCK - 1)
                            nc.vector.tensor_add(
                                v_sb[:, t * 4 + s, dk * 512:(dk + 1) * 512],
                                ps[:, 0:512],
                                bv_rep[:, dk * 512:(dk + 1) * 512],
                            )

                # ---- Phase A+C fused per Lq tile ----
                for t in range(NCQ):
                    qst = qsp.tile([128, KCQ, c_t], bf16, tag="qst")
                    nc.sync.dma_start(qst, queryT[b, t])
                    qTt = qTp.tile([128, KCK, c_t], bf16, tag="qTt")
                    for mc in range(KCK):
                        ps = ps512.tile([128, c_t], fp32, tag="mm512")
                        for kc in range(KCQ):
                            mm(ps, wq_sb[:, kc, mc * 128:(mc + 1) * 128],
                               qst[:, kc, :], kc == 0, kc == KCQ - 1)
                        nc.vector.tensor_scalar_add(
                            qTt[:, mc, :], ps, bq_sb[:, mc:mc + 1]
                        )
                    exps = []
                    for lkb in range(NLK):
                        ps_s = ps512.tile([128, c_t], fp32, tag="mm512")
                        for kc in range(KCK):
                            mm(ps_s, kT_sb[:, kc, lkb * 128:(lkb + 1) * 128],
                               qTt[:, kc, :], kc == 0, kc == KCK - 1)
                        ex = ep.tile([128, c_t], bf16, tag="exp")
                        nc.scalar.activation(
                            ex, ps_s, mybir.ActivationFunctionType.Exp,
                            scale=1.0 / 32.0,
                        )
                        exps.append(ex)
                    for s in range(CS):
                        ps_o = ps_op.tile([128, DK], fp32, tag="pv")
                        ps_n = ps_np.tile([128, 4], fp32, tag="sum")
                        for lkb in range(NLK):
                            lhs = exps[lkb][:, s * 128:(s + 1) * 128]
                            for dk in range(2):
                                mm(ps_o[:, dk * 512:(dk + 1) * 512], lhs,
                                   v_sb[:, lkb, dk * 512:(dk + 1) * 512],
                                   lkb == 0, lkb == NLK - 1)
                            mm(ps_n, lhs, ones_col, lkb == 0, lkb == NLK - 1)
                        rec = op.tile([128, 1], fp32, tag="rec")
                        nc.vector.reciprocal(rec, ps_n[:, 0:1])
                        o_sb = op.tile([128, DK], bf16, tag="osb")
                        nc.scalar.activation(
                            o_sb, ps_o,
                            mybir.ActivationFunctionType.Copy, scale=rec,
                        )
                        nc.sync.dma_start(
                            out[b, t * c_t + s * 128: t * c_t + (s + 1) * 128, :],
                            o_sb,
                        )
    return nc


_NC_CACHE = {}


def _get_nc(key=("v3",)):
    if key not in _NC_CACHE:
        _NC_CACHE[key] = build_nc()
    return _NC_CACHE[key]


def make_in_maps(inputs):
    """Host prep: cast bf16, pack SBUF-image layouts, shard by batch."""
    import ml_dtypes

    bf16 = ml_dtypes.bfloat16
    f32c = lambda x: np.ascontiguousarray(np.asarray(x), dtype=np.float32)

    # query [B, lq, dq] -> [B, t, p, kc, n]: lq = t*512+n, dq = kc*128+p
    qT = np.ascontiguousarray(
        np.asarray(inputs["query"]).astype(bf16)
        .reshape(B, NCQ, C_T, KCQ, 128).transpose(0, 1, 4, 3, 2)
    )
    kT = np.ascontiguousarray(
        np.asarray(inputs["key"]).astype(bf16)
        .reshape(B, NTK, 512, KCK, 128).transpose(0, 1, 4, 3, 2)
    )
    # W [dk_in, dk_out] -> [p, kc, dk_out]: dk_in = kc*128+p
    def w_img(w, kc):
        return np.ascontiguousarray(
            np.asarray(w).astype(bf16).reshape(kc, 128, -1).transpose(1, 0, 2)
        )

    shared = {
        "Wq": w_img(inputs["Wq"], KCQ),
        "Wk": w_img(inputs["Wk"], KCK),
        "Wv": w_img(inputs["Wv"], KCK),
        "bq": f32c(inputs["bq"]),
        "bv": f32c(inputs["bv"]),
    }
    in_maps = []
    for c in range(N_CORES):
        m = dict(shared)
        m["queryT"] = qT[c * BPC:(c + 1) * BPC]
        m["keyT"] = kT[c * BPC:(c + 1) * BPC]
        in_maps.append(m)
    return in_maps


def kernel(**inputs):
    from concourse.bass_utils import run_bass_kernel_spmd

    nc = _get_nc()
    if not nc.is_finalized():
        nc.finalize()
    in_maps = make_in_maps(inputs)
    res = run_bass_kernel_spmd(nc, in_maps, core_ids=list(range(N_CORES)))
    return np.concatenate(
        [r["out"].astype(np.float32) for r in res.results], axis=0
    )


# revision 12
# speedup vs baseline: 1.6296x; 1.0701x over previous
"""CrossAttention Trainium2 Bass kernel (v4).

Problem (hardcoded): B=16, Lq=Lk=2048, Dq=768, Dk=1024, fp32.
  q = query @ Wq + bq ; k = key @ Wk + bk ; v = key @ Wv + bv
  out = softmax(q k^T / sqrt(1024)) @ v

Sharding: data-parallel over batch, 2 batches per core on 8 cores.

Math simplifications (exact up to rounding):
  - bk shifts every score row by a per-query constant -> cancels in softmax.
  - bv folds into v (softmax weights sum to 1): v' = k@Wv + bv gives
    out = (sum exp * v') / sum exp directly.
  - scores are bounded (|s|/32 < ~3) so exp() without max-subtraction is safe.

v3 (vs v2): all DMAs use host-prepacked SBUF-image layouts (128 fat
contiguous descriptors per transfer instead of ~1024 thin ones), cutting
DMA-issue serialization at kernel start; weight DMAs issue in phase order
(Wk, Wv first); output is downloaded bf16 and cast to fp32 on the host.
Device work per batch: kT = Wk^T keyT and v = keyT^T Wv + bv resident in
SBUF, then per 512-col Lq tile: qT = Wq^T queryT + bq, scoresT = kT^T qT,
exp(s/32), PV + ones-column row sums, out = PV * (1/sums).
"""

import numpy as np

B, LQ, LK = 16, 2048, 2048
DQ, DK = 768, 1024
N_CORES = 8
BPC = B // N_CORES  # batches per core

KCQ = DQ // 128  # 6 contraction chunks for q projection
KCK = DK // 128  # 8 contraction chunks for k/v projection + scores
NLK = LK // 128  # 16 Lk subtiles of 128
C_T = 512
NCQ = LQ // C_T  # Lq tiles
NTK = LK // 512  # Lk tiles (projection phase)


def build_nc(bpc=BPC, lq=LQ, lk=LK, c_t=C_T):
    import concourse.bass as bass
    import concourse.mybir as mybir
    from concourse import bacc
    import concourse.tile as tile

    fp32 = mybir.dt.float32
    bf16 = mybir.dt.bfloat16
    CS = c_t // 128  # Lq subtiles per attention tile

    nc = bacc.Bacc("TRN2")
    # Host-prepacked SBUF-image layouts: partition dim explicit, per-partition
    # slabs contiguous so every DMA is 128 fat descriptors.
    queryT = nc.dram_tensor(
        "queryT", [bpc, NCQ, 128, KCQ, c_t], bf16, kind="ExternalInput")
    keyT = nc.dram_tensor(
        "keyT", [bpc, NTK, 128, KCK, 512], bf16, kind="ExternalInput")
    Wq = nc.dram_tensor("Wq", [128, KCQ, DK], bf16, kind="ExternalInput")
    Wk = nc.dram_tensor("Wk", [128, KCK, DK], bf16, kind="ExternalInput")
    Wv = nc.dram_tensor("Wv", [128, KCK, DK], bf16, kind="ExternalInput")
    bq = nc.dram_tensor("bq", [DK], fp32, kind="ExternalInput")
    bv = nc.dram_tensor("bv", [DK], fp32, kind="ExternalInput")
    out = nc.dram_tensor("out", [bpc, lq, DK], bf16, kind="ExternalOutput")

    fp8 = mybir.dt.float8e4
    DR = mybir.MatmulPerfMode.DoubleRow

    def mm(ps, lhsT, rhs, start, stop, perf_mode=None):
        nc.tensor.matmul(ps, lhsT, rhs, start=start, stop=stop,
                         perf_mode=perf_mode)

    with tile.TileContext(nc) as tc:
        with (
            tc.tile_pool(name="const", bufs=1) as constp,
            tc.tile_pool(name="w", bufs=1) as wp,
            tc.tile_pool(name="kT", bufs=1) as kTp,
            tc.tile_pool(name="v", bufs=1) as vp,
            tc.tile_pool(name="kstage", bufs=2) as ksp,
            tc.tile_pool(name="qstage", bufs=2) as qsp,
            tc.tile_pool(name="qT", bufs=2) as qTp,
            tc.tile_pool(name="exp", bufs=NLK + 2) as ep,
            tc.tile_pool(name="osb", bufs=2) as op,
            tc.tile_pool(name="ps512", bufs=2, space="PSUM") as ps512,
            tc.tile_pool(name="ps_o", bufs=2, space="PSUM") as ps_op,
            tc.tile_pool(name="ps_n", bufs=2, space="PSUM") as ps_np,
        ):
            # Phase-order DMA issue: Wk halves first (first matmuls), then Wv
            # (needed ~10us in), bv (v evacuation), then Wq/bq (phase A).
            # Interleave Wk and the first key tile so the first accumulation
            # group's operands (kc 0-3 of both) land earliest; the kc 4-7
            # halves stream in during the first matmul group.
            wk_sb = wp.tile([128, KCK, DK], bf16)
            kst0 = ksp.tile([128, KCK, 512], bf16, tag="kst")
            nc.sync.dma_start(wk_sb[:, 0:4, :], Wk[:, 0:4, :])
            nc.sync.dma_start(kst0[:, 0:4, :], keyT[0, 0, :, 0:4, :])
            nc.sync.dma_start(wk_sb[:, 4:8, :], Wk[:, 4:8, :])
            nc.sync.dma_start(kst0[:, 4:8, :], keyT[0, 0, :, 4:8, :])
            wv_sb = wp.tile([128, KCK, DK], bf16)
            nc.sync.dma_start(wv_sb, Wv[:, :, :])
            bv_rep = constp.tile([128, DK], fp32)
            nc.sync.dma_start(bv_rep, bv[None, :].partition_broadcast(128))
            wq_sb = wp.tile([128, KCQ, DK], bf16)
            nc.sync.dma_start(wq_sb, Wq[:, :, :])
            bq_sb = constp.tile([128, KCK], fp32)
            nc.sync.dma_start(bq_sb, bq.rearrange("(c p) -> p c", p=128))
            ones_f32 = constp.tile([128, 4], fp32)
            nc.vector.memset(ones_f32, 1.0)
            ones_col = constp.tile([128, 4], bf16)
            nc.vector.tensor_copy(ones_col, ones_f32)

            for b in range(bpc):
                # Hybrid-precision scores: dk chunks 0-3 of kT/qT in fp8
                # (DoubleRow pairs, ~2x PE rate), chunks 4-7 in bf16.
                kT8_sb = kTp.tile([128, 4, lk], fp8, tag="kT8")   # [dk0:512, lk]
                kT_sb = kTp.tile([128, 4, lk], bf16, tag="kT")    # [dk512:, lk]
                v_sb = vp.tile([128, NLK, DK], bf16, tag="v")     # [lk, dk]

                # ---- Phase B: kT and v (+bv), SBUF resident ----
                for t in range(NTK):
                    if b == 0 and t == 0:
                        kst = kst0
                    else:
                        kst = ksp.tile([128, KCK, 512], bf16, tag="kst")
                        nc.sync.dma_start(kst[:, 0:4, :], keyT[b, t, :, 0:4, :])
                        nc.sync.dma_start(kst[:, 4:8, :], keyT[b, t, :, 4:8, :])
                    for mc in range(KCK):
                        ps = ps512.tile([128, 512], fp32, tag="mm512")
                        for kc in range(KCK):
                            mm(ps, wk_sb[:, kc, mc * 128:(mc + 1) * 128],
                               kst[:, kc, :], kc == 0, kc == KCK - 1)
                        if mc < 4:
                            nc.vector.tensor_copy(
                                kT8_sb[:, mc, t * 512:(t + 1) * 512], ps
                            )
                        else:
                            nc.vector.tensor_copy(
                                kT_sb[:, mc - 4, t * 512:(t + 1) * 512], ps
                            )
                    for s in range(4):
                        for dk in range(2):
                            ps = ps_op.tile([128, DK], fp32, tag="pv")
                            for kc in range(KCK):
                                mm(ps[:, 0:512],
                                   kst[:, kc, s * 128:(s + 1) * 128],
                                   wv_sb[:, kc, dk * 512:(dk + 1) * 512],
                                   kc == 0, kc == KCK - 1)
                            nc.vector.tensor_add(
                                v_sb[:, t * 4 + s, dk * 512:(dk + 1) * 512],
                                ps[:, 0:512],
                                bv_rep[:, dk * 512:(dk + 1) * 512],
                            )

                # ---- Phase A+C fused per Lq tile ----
                for t in range(NCQ):
                    qst = qsp.tile([128, KCQ, c_t], bf16, tag="qst")
                    nc.sync.dma_start(qst, queryT[b, t])
                    qT8 = qTp.tile([128, 4, c_t], fp8, tag="qT8")
                    qTt = qTp.tile([128, 4, c_t], bf16, tag="qTt")
                    for mc in range(KCK):
                        ps = ps512.tile([128, c_t], fp32, tag="mm512")
                        for kc in range(KCQ):
                            mm(ps, wq_sb[:, kc, mc * 128:(mc + 1) * 128],
                               qst[:, kc, :], kc == 0, kc == KCQ - 1)
                        if mc < 4:
                            nc.vector.tensor_scalar_add(
                                qT8[:, mc, :], ps, bq_sb[:, mc:mc + 1]
                            )
                        else:
                            nc.vector.tensor_scalar_add(
                                qTt[:, mc - 4, :], ps, bq_sb[:, mc:mc + 1]
                            )
                    exps = []
                    for lkb in range(NLK):
                        ps_s = ps512.tile([128, c_t], fp32, tag="mm512")
                        lks = slice(lkb * 128, (lkb + 1) * 128)
                        mm(ps_s, kT8_sb[:, 0:2, lks], qT8[:, 0:2, :],
                           True, False, perf_mode=DR)
                        mm(ps_s, kT8_sb[:, 2:4, lks], qT8[:, 2:4, :],
                           False, False, perf_mode=DR)
                        for kc in range(4):
                            mm(ps_s, kT_sb[:, kc, lks],
                               qTt[:, kc, :], False, kc == 3)
                        ex = ep.tile([128, c_t], bf16, tag="exp")
                        nc.scalar.activation(
                            ex, ps_s, mybir.ActivationFunctionType.Exp,
                            scale=1.0 / 32.0,
                        )
                        exps.append(ex)
                    for s in range(CS):
                        ps_o = ps_op.tile([128, DK], fp32, tag="pv")
                        ps_n = ps_np.tile([128, 4], fp32, tag="sum")
                        for lkb in range(NLK):
                            lhs = exps[lkb][:, s * 128:(s + 1) * 128]
                            for dk in range(2):
                                mm(ps_o[:, dk * 512:(dk + 1) * 512], lhs,
                                   v_sb[:, lkb, dk * 512:(dk + 1) * 512],
                                   lkb == 0, lkb == NLK - 1)
                            mm(ps_n, lhs, ones_col, lkb == 0, lkb == NLK - 1)
                        rec = op.tile([128, 1], fp32, tag="rec")
                        nc.vector.reciprocal(rec, ps_n[:, 0:1])
                        o_sb = op.tile([128, DK], bf16, tag="osb")
                        nc.scalar.activation(
                            o_sb, ps_o,
                            mybir.ActivationFunctionType.Copy, scale=rec,
                        )
                        nc.sync.dma_start(
                            out[b, t * c_t + s * 128: t * c_t + (s + 1) * 128, :],
                            o_sb,
                        )
    return nc


_NC_CACHE = {}


def _get_nc(key=("v3",)):
    if key not in _NC_CACHE:
        _NC_CACHE[key] = build_nc()
    return _NC_CACHE[key]


def make_in_maps(inputs):
    """Host prep: cast bf16, pack SBUF-image layouts, shard by batch."""
    import ml_dtypes

    bf16 = ml_dtypes.bfloat16
    f32c = lambda x: np.ascontiguousarray(np.asarray(x), dtype=np.float32)

    # query [B, lq, dq] -> [B, t, p, kc, n]: lq = t*512+n, dq = kc*128+p
    qT = np.ascontiguousarray(
        np.asarray(inputs["query"]).astype(bf16)
        .reshape(B, NCQ, C_T, KCQ, 128).transpose(0, 1, 4, 3, 2)
    )
    kT = np.ascontiguousarray(
        np.asarray(inputs["key"]).astype(bf16)
        .reshape(B, NTK, 512, KCK, 128).transpose(0, 1, 4, 3, 2)
    )
    # W [dk_in, dk_out] -> [p, kc, dk_out]: dk_in = kc*128+p
    def w_img(w, kc):
        return np.ascontiguousarray(
            np.asarray(w).astype(bf16).reshape(kc, 128, -1).transpose(1, 0, 2)
        )

    shared = {
        "Wq": w_img(inputs["Wq"], KCQ),
        "Wk": w_img(inputs["Wk"], KCK),
        "Wv": w_img(inputs["Wv"], KCK),
        "bq": f32c(inputs["bq"]),
        "bv": f32c(inputs["bv"]),
    }
    in_maps = []
    for c in range(N_CORES):
        m = dict(shared)
        m["queryT"] = qT[c * BPC:(c + 1) * BPC]
        m["keyT"] = kT[c * BPC:(c + 1) * BPC]
        in_maps.append(m)
    return in_maps


def kernel(**inputs):
    from concourse.bass_utils import run_bass_kernel_spmd

    nc = _get_nc()
    if not nc.is_finalized():
        nc.finalize()
    in_maps = make_in_maps(inputs)
    res = run_bass_kernel_spmd(nc, in_maps, core_ids=list(range(N_CORES)))
    return np.concatenate(
        [r["out"].astype(np.float32) for r in res.results], axis=0
    )
